# revision 10
# baseline (speedup 1.0000x reference)
"""Trainium2 Bass kernel for nn_NoOtherStartingStatesLoss.

loss = sum_k [ col_sums[ss_k] - diag_sums[ss_k] ]  over softmax(T, axis=-1)

Per row (t, s1) only the softmax denominator d = sum_s2 exp(T[t,s1,s2]) and
the 4 gathered numerators exp(T[t,s1,ss_k]) are needed, so the full softmax
is never materialized to HBM.  T is sharded over the transitions axis across
8 NeuronCores; each core reduces its slab to a partial scalar; host sums the
8 partials.

Numerics: |T| <= ~6.5 for N(0,1) fills, so exp() without the max-subtraction
is safe in fp32 and matches jax.nn.softmax to ~1e-6.

Schedule: the kernel is DMA-bound (~135 MB @ ~360 GB/s/core).  The diagonal
correction rows are loaded first so they overlap the main stream, and the
final tiles are tapered (smaller J) so the exp+reduce pipeline tail after
the last DMA is short.
"""

import sys

sys.path.insert(0, "/opt/trn_rl_repo")

import numpy as np
from contextlib import ExitStack

N_TRANS, N_STATES, N_START = 1024, 512, 4
N_CORES = 8
T_PC = N_TRANS // N_CORES  # transitions per core

_PROGRAM_CACHE = {}

# schedule knobs (resolved at build time; cache key includes them)
CONFIG = {
    "diag_at": 4,       # emit diag pass after this many main tiles
    "accum_mod": 1,     # big tile i uses ACT-accum when i % 4 == accum_mod
    "tail_accum": False,  # tapered tail tiles use ACT-accum mode
    "dual_queue": False,  # alternate big-tile DMAs across sync/scalar HWDGE rings
                          # (measured worse: scalar-ring dispatch interferes w/ ACT)
}


def _tile_plan(rows_total):
    """List of (row_start, J) with J rows per partition per tile."""
    plan = []
    row = 0
    # main tiles: J=16 (4 MiB each), tail tapered to shorten the pipe drain
    tail = [4, 4, 4, 4]
    tail_rows = sum(128 * j for j in tail)
    while row < rows_total - tail_rows:
        plan.append((row, 16))
        row += 128 * 16
    assert row == rows_total - tail_rows
    for j in tail:
        plan.append((row, j))
        row += 128 * j
    assert row == rows_total
    return plan


def _build_program(t_pc, ss, diag_at, accum_mod, tail_accum, dual_queue):
    """Build + compile the per-core Bass program.

    t_pc: transitions in this core's slab.
    ss: tuple of 4 starting-state column indices (python ints).
    """
    import concourse.bacc as bacc
    import concourse.tile as tile
    import concourse.mybir as mybir

    f32 = mybir.dt.float32
    EXP = mybir.ActivationFunctionType.Exp
    AX = mybir.AxisListType.X

    rows_total = t_pc * N_STATES
    plan = _tile_plan(rows_total)
    acc_w = sum(j for _, j in plan)
    assert t_pc <= 128

    nc = bacc.Bacc("TRN2", target_bir_lowering=False, debug=False,
                   enable_asserts=False, num_devices=N_CORES)
    t_in = nc.dram_tensor("t_slab", [rows_total, N_STATES], f32,
                          kind="ExternalInput")
    out_d = nc.dram_tensor("partial", [1, 1], f32, kind="ExternalOutput")
    t_ap = t_in.ap()
    # [t, s1, s2] view for the diagonal-row gather
    t_rows = t_ap.rearrange("(t a) s -> t a s", a=N_STATES)

    with tile.TileContext(nc) as tc, ExitStack() as ctx:
        xp = ctx.enter_context(tc.tile_pool(name="x", bufs=5))
        dk_pool = ctx.enter_context(tc.tile_pool(name="dk", bufs=4))
        sp = ctx.enter_context(tc.tile_pool(name="small", bufs=6))
        accp = ctx.enter_context(tc.tile_pool(name="acc", bufs=1))
        pp = ctx.enter_context(tc.tile_pool(name="ps", bufs=1, space="PSUM"))

        # one slot per (tile, row-in-partition): no cross-tile serialization
        accw = accp.tile([128, acc_w], f32)
        corr = accp.tile([128, N_START], f32)

        def diag_pass():
            # diagonal-correction rows; emitted just after the first few
            # main tiles so the strided DMAs overlap the stream without
            # delaying tile 0's dispatch
            nc.vector.memset(corr[:], 0.0)
            for k in range(N_START):
                dk = dk_pool.tile([t_pc, N_STATES], f32, tag="dk")
                nc.sync.dma_start(out=dk[:], in_=t_rows[:, ss[k], :])
                dsum = sp.tile([t_pc, 1], f32, tag="dsum")
                nc.scalar.activation(dk[:], dk[:], EXP, accum_out=dsum[:])
                rk = sp.tile([t_pc, 1], f32, tag="rk")
                nc.vector.reciprocal(rk[:], dsum[:])
                nc.vector.tensor_mul(corr[:t_pc, k:k + 1],
                                     dk[:, ss[k]:ss[k] + 1], rk[:])

        # ---- main pass: softmax denominators + 4-column numerators ----
        # DVE's grouped reduce_sum (8.7us/tile) runs just at the DMA rate
        # (9.9us/tile), so every 4th big tile computes its denominators on
        # ACT instead (exp chunked per row-group with accum_out); both
        # engines then average well under the DMA streaming rate.
        col = 0
        if diag_at == 0:
            diag_pass()
        for i, (row0, J) in enumerate(plan):
            if i == diag_at and i > 0:
                diag_pass()
            use_accum = ((i % 4 == accum_mod) and J == 16) or \
                        (tail_accum and J < 16)
            x = xp.tile([128, J * N_STATES], f32, tag="x")
            src = t_ap[row0:row0 + 128 * J, :] \
                .rearrange("(p j) s -> p (j s)", p=128)
            dma_eng = nc.scalar if (dual_queue and i % 2 == 1) else nc.sync
            dma_eng.dma_start(out=x[:, :J * N_STATES], in_=src)
            x3 = x[:, :J * N_STATES].rearrange("p (j s) -> p j s",
                                               s=N_STATES)
            d = sp.tile([128, J], f32, tag="d")
            if use_accum:
                for j in range(J):
                    nc.scalar.activation(x3[:, j, :], x3[:, j, :], EXP,
                                         accum_out=d[:, j:j + 1])
            else:
                nc.scalar.activation(x[:, :J * N_STATES],
                                     x[:, :J * N_STATES], EXP)  # in-place
                nc.vector.reduce_sum(d[:, :J], x3, axis=AX)
            r = sp.tile([128, J], f32, tag="r")
            nc.vector.reciprocal(r[:, :J], d[:, :J])
            n = sp.tile([128, J], f32, tag="n")
            n2 = sp.tile([128, J], f32, tag="n2")
            nc.vector.tensor_add(n[:, :J], x3[:, :, ss[0]], x3[:, :, ss[1]])
            nc.vector.tensor_add(n2[:, :J], x3[:, :, ss[2]], x3[:, :, ss[3]])
            nc.vector.tensor_add(n[:, :J], n[:, :J], n2[:, :J])
            nc.vector.tensor_mul(accw[:, col:col + J], n[:, :J], r[:, :J])
            col += J
        assert col == acc_w

        # ---- fold to scalar ----
        msum = sp.tile([128, 1], f32)
        nc.vector.reduce_sum(msum[:], accw[:], axis=AX)
        csum = sp.tile([128, 1], f32)
        nc.vector.reduce_sum(csum[:], corr[:], axis=AX)
        fin = sp.tile([128, 1], f32)
        nc.vector.tensor_sub(fin[:], msum[:], csum[:])
        ones = accp.tile([128, 1], f32)
        nc.vector.memset(ones[:], 1.0)
        ps = pp.tile([1, 1], f32)
        nc.tensor.matmul(ps[:], ones[:], fin[:], start=True, stop=True)
        ot = sp.tile([1, 1], f32)
        nc.vector.tensor_copy(ot[:], ps[:])
        nc.sync.dma_start(out=out_d.ap(), in_=ot[:])

    nc.compile()
    return nc


def kernel(T, starting_states, _trace=False):
    from concourse.bass_utils import run_bass_kernel_spmd

    T = np.ascontiguousarray(np.asarray(T, dtype=np.float32))
    ss = tuple(int(s) for s in np.asarray(starting_states).ravel())
    assert T.shape == (N_TRANS, N_STATES, N_STATES)
    assert len(ss) == N_START

    key = (T_PC, ss, CONFIG["diag_at"], CONFIG["accum_mod"],
           CONFIG["tail_accum"], CONFIG["dual_queue"])
    if key not in _PROGRAM_CACHE:
        _PROGRAM_CACHE[key] = _build_program(*key)
    nc = _PROGRAM_CACHE[key]

    rows_total = T_PC * N_STATES
    in_maps = [
        {"t_slab": T[c * T_PC:(c + 1) * T_PC].reshape(rows_total, N_STATES)}
        for c in range(N_CORES)
    ]
    res = run_bass_kernel_spmd(nc, in_maps, core_ids=list(range(N_CORES)),
                               trace=_trace)
    kernel.last_result = res
    total = np.float64(0.0)
    for c in range(N_CORES):
        total += np.float64(res.results[c]["partial"][0, 0])
    return np.asarray(total, dtype=np.float32)
